# revision 8
# baseline (speedup 1.0000x reference)
"""Trainium2 Bass kernel for nn_CerberusSemanticIDBranch (vq_codebook).

Reference semantics (per group g with prototypes P_g [K_g, D]):
    xn = x / (||x|| + 1e-6)
    logits = (xn @ l2(P_g).T) / tau
    q = softmax(logits)
    q_aff = q @ A_g;  q_aff /= (q_aff.sum(-1) + 1e-6)
    out_g = q_aff @ P_g
stacked over 5 groups -> [B, 5, D].

Host folds: A_g row-sums are constant -> out_g = q @ W_g with
W_g = (A_g/(c_g+1e-6)) @ P_g; l2(P) precomputed; 1/(tau*||x||) applied
as a per-batch scale S on the logits.

v2 (PE/DMA rework vs baseline):
  * bf16 streams: xt input, prototype/W constants, et/qt intermediates
    and the output store are bf16 (host upcasts the result to f32).
    Halves HBM traffic and turns the 4-cyc/col f32 matmuls (seg/invb)
    into 1-cyc/col bf16 ones.
  * ssq via a single PE matmul on pre-summed squares (chunk squares on
    gpsimd, chunk adds on DVE): 2048 -> 512 PE cycles per supertile.
  * S broadcast matmul in f32r (1 cyc/col), knob to f32.
  * mm2 uses tile_position so the small-K group matmuls share the PE
    array: A-layout pair at rows (0,64), B-layout triple at (0,32,64).
  * raw/invB PSUM tiles merged per layout-pair ([128,2,512]); PSUM
    pools rebalanced small(1)+mid(2x2)+out(3x1) = 8 banks.
  * input DMA on the SP HWDGE ring, output DMA on the ACT ring; one
    merged output DMA per 512-row supertile.

Data parallel over 8 NeuronCores: core i handles rows [i*4096, (i+1)*4096).
"""

import itertools
import sys

import numpy as np

sys.path.insert(0, "/opt/trn_rl_repo")

import ml_dtypes  # noqa: E402

import concourse.bass as bass  # noqa: E402
import concourse.tile as tile  # noqa: E402
from concourse import mybir  # noqa: E402
from concourse.vector_clock import ScopedClock  # noqa: E402

# ---------------------------------------------------------------- problem
GROUP_DIMS = {
    "gender": [2],
    "hair": [5, 3],
    "top": [8, 5],
    "pants": [8, 5],
    "shoes": [6, 4],
}
TAU = 0.07
B, D = 32768, 512
N_CORES = 8
B_CORE = B // N_CORES          # 4096
SUPER = 512                    # batch rows per supertile
N_SUPER = B_CORE // SUPER      # 8
N_CHUNK = D // 128             # 4
N_GROUPS = 5

# group -> (K, layout, base partition); layout A=0, B=1
GROUP_PLACEMENT = {
    "gender": (2, 1, 0),
    "hair": (15, 1, 32),
    "top": (40, 0, 0),
    "pants": (40, 0, 64),
    "shoes": (24, 1, 64),
}
# mm2 issue order: A-layout pair then B-layout triple (tile_position
# concurrency within each set: disjoint 32-aligned PE array rows).
MM2_ORDER = ["top", "pants", "gender", "hair", "shoes"]

# bf16 const blob column offsets ([128, CB_COLS])
_PNT_OFF = 0                       # 4 chunks x (A 128 | B 128) = 1024
_W_OFF = 1024                      # 2 layouts x 512
_IND_OFF = 2048                    # 2 layouts x 5
_INDT_OFF = 2058                   # 2 layouts x 128 (rows 0:5)
_ONE_OFF = 2314                    # [128, 1] ones
CB_COLS = 2315
CF_COLS = 128                      # f32 ones blob (row 0 = ones_row)

MM2_TILE_POS = True
BCAST_DT = "f32r"                  # "f32r" (1 cyc/col) or "f32" (exact)

_F32 = mybir.dt.float32
_F32R = mybir.dt.float32r
_BF16 = mybir.dt.bfloat16
_EXP = mybir.ActivationFunctionType.Exp
_LN = mybir.ActivationFunctionType.Ln
_NP_BF16 = ml_dtypes.bfloat16


# ------------------------------------------------------------- tile patch
_NOP_ID = [0]


def _spread_all_waits(nc, max_waits=1):
    """This walrus build rejects instructions carrying more than one sync
    wait (setupSyncWait: "Too many sync wait commands").  Rewrite every
    block so extra waits ride on dedicated same-engine NOPs placed just
    before the instruction (engine queues are FIFO, so semantics hold)."""
    for fn in nc.m.functions:
        for blk in fn.blocks:
            insts = list(blk.instructions)
            out = []
            changed = False
            for inst in insts:
                si = inst.sync_info
                waits = list(si.on_wait) if si is not None and si.on_wait else []
                if len(waits) > max_waits:
                    changed = True
                    for w in waits[:-max_waits]:
                        _NOP_ID[0] += 1
                        out.append(
                            mybir.InstNoOp(
                                name=f"waitnop-{_NOP_ID[0]}",
                                engine=inst.engine,
                                bass_nofuse=True,
                                sync_info=mybir.SyncInfo(
                                    on_wait=[w], on_update=[]),
                            ))
                    si.on_wait = waits[-max_waits:]
                out.append(inst)
            if changed:
                blk.instructions = out


def _patched_drain_and_barrier(self, tick_clock, wait_clock):
    probe = self.nc.sync.nop(nofuse=True)
    wait_clock.add_sem_waits(probe.ins, ScopedClock({None: tick_clock.global_clock}))
    drain_inst = self.nc.sync.drain()
    del drain_inst
    self.nc.all_engine_barrier()
    assert self.sems is not None
    popped = self.nc._tile_sem_poison_stack.pop()
    assert popped is self._sem_poison
    self.nc.clear_and_free_semaphores(list(self.sems.allocated().values()))
    self.nc.all_engine_barrier()
    _spread_all_waits(self.nc)


_patched = False


def _install_tile_patch():
    global _patched
    if not _patched:
        tile.TileContext._drain_and_barrier = _patched_drain_and_barrier
        _patched = True


# --------------------------------------------------------- host constants
def _affinity(dims):
    combos = np.array(
        list(itertools.product(*[range(d) for d in dims])), dtype=np.int32
    )
    return (combos[:, None, :] == combos[None, :, :]).mean(-1).astype(np.float64)


def build_host_constants(protos):
    """protos: dict name -> P_g [K_g, D] float32. Returns (cb bf16
    [128, CB_COLS], cf f32 [128, CF_COLS]) shared by all cores."""
    pn_pad = np.zeros((2, 128, D), dtype=np.float32)   # l2-normalized, padded
    w_pad = np.zeros((2, 128, D), dtype=np.float32)    # (A/c) @ P, padded
    ind = np.zeros((2, 128, N_GROUPS), dtype=np.float32)
    indt = np.zeros((2, N_GROUPS, 128), dtype=np.float32)

    for g, name in enumerate(GROUP_DIMS):
        P = np.asarray(protos[name], dtype=np.float32)
        K, layout, base = GROUP_PLACEMENT[name]
        assert P.shape == (K, D)
        norm = np.linalg.norm(P, axis=-1, keepdims=True).astype(np.float32)
        pn = P / (norm + np.float32(1e-6))
        A = _affinity(GROUP_DIMS[name])                 # [K, K] float64
        c = A[0].sum() + 1e-6                            # constant row sum
        W = ((A / c) @ P.astype(np.float64)).astype(np.float32)
        pn_pad[layout, base : base + K] = pn
        w_pad[layout, base : base + K] = W
        ind[layout, base : base + K, g] = 1.0
        indt[layout, g, base : base + K] = 1.0

    cb = np.zeros((128, CB_COLS), dtype=np.float32)
    for c in range(N_CHUNK):
        for l in range(2):
            cb[:, _PNT_OFF + c * 256 + l * 128 : _PNT_OFF + c * 256 + (l + 1) * 128] = (
                pn_pad[l][:, c * 128 : (c + 1) * 128].T
            )
    for l in range(2):
        cb[:, _W_OFF + l * D : _W_OFF + (l + 1) * D] = w_pad[l]
        cb[:, _IND_OFF + l * N_GROUPS : _IND_OFF + (l + 1) * N_GROUPS] = ind[l]
        cb[0:N_GROUPS, _INDT_OFF + l * 128 : _INDT_OFF + (l + 1) * 128] = indt[l]
    cb[:, _ONE_OFF] = 1.0
    cf = np.ones((128, CF_COLS), dtype=np.float32)
    return cb.astype(_NP_BF16), cf


# ------------------------------------------------------------ bass program
def build_program(loop_k=None, ablate=None, repeat=1):
    """Emit the SPMD program. loop_k: if set, wrap the body in a tc.For_i
    repeat for delta-timing. repeat: python-unrolled repeats. ablate:
    None | "dma_only" | "no_outdma" (perf diagnostics; wrong results)."""
    _install_tile_patch()
    nc = bass.Bass("TRN2", target_bir_lowering=False, debug=False,
                   num_devices=N_CORES)
    xt_d = nc.dram_tensor("xt", [D, B_CORE], _BF16, kind="ExternalInput").ap()
    cb_d = nc.dram_tensor("cb", [128, CB_COLS], _BF16,
                          kind="ExternalInput").ap()
    cf_dt = _F32R if BCAST_DT == "f32r" else _F32
    cf_d = nc.dram_tensor("cf", [128, CF_COLS], cf_dt,
                          kind="ExternalInput").ap()
    out_d = nc.dram_tensor("out", [B_CORE, N_GROUPS, D], _BF16,
                           kind="ExternalOutput").ap()

    with tile.TileContext(nc) as tc:
        import contextlib

        with contextlib.ExitStack() as ctx:
            cpool = ctx.enter_context(tc.tile_pool(name="consts", bufs=1))
            xt_pool = ctx.enter_context(tc.tile_pool(name="xt", bufs=4))
            sq4_pool = ctx.enter_context(tc.tile_pool(name="sq4", bufs=2))
            sqt_pool = ctx.enter_context(tc.tile_pool(name="sqt", bufs=4))
            tiny = ctx.enter_context(tc.tile_pool(name="tiny", bufs=6))
            sp_pool = ctx.enter_context(tc.tile_pool(name="sp", bufs=2))
            rw_pool = ctx.enter_context(tc.tile_pool(name="rw", bufs=2))
            et_pool = ctx.enter_context(tc.tile_pool(name="et", bufs=3))
            qt_pool = ctx.enter_context(tc.tile_pool(name="qt", bufs=3))
            stage = ctx.enter_context(tc.tile_pool(name="stage", bufs=3))
            ps_small = ctx.enter_context(
                tc.tile_pool(name="ps_small", bufs=1, space="PSUM"))
            ps_mid = ctx.enter_context(
                tc.tile_pool(name="ps_mid", bufs=2, space="PSUM"))
            ps_out = ctx.enter_context(
                tc.tile_pool(name="ps_out", bufs=3, space="PSUM"))

            cb = cpool.tile([128, CB_COLS], _BF16)
            nc.sync.dma_start(out=cb[:], in_=cb_d[:])
            cf = cpool.tile([128, CF_COLS], cf_dt)
            nc.sync.dma_start(out=cf[:], in_=cf_d[:])

            def pnt(c, l):
                o = _PNT_OFF + c * 256 + l * 128
                return cb[:, o : o + 128]

            def w_l(l, base, K):
                return cb[base : base + K, _W_OFF + l * D : _W_OFF + (l + 1) * D]

            def ind_l(l):
                o = _IND_OFF + l * N_GROUPS
                return cb[:, o : o + N_GROUPS]

            def indt_l(l):
                o = _INDT_OFF + l * 128
                return cb[0:N_GROUPS, o : o + 128]

            ones_col = cb[:, _ONE_OFF : _ONE_OFF + 1]          # [128,1] bf16
            ones_row = cf[0:1, 0:128]                          # [1,128]

            groups = [(name, GROUP_PLACEMENT[name]) for name in MM2_ORDER]
            # evac engine schedule: DVE-heavy (245G vs 153G elem/s);
            # gpsimd cannot read PSUM so it only gets SBUF-side work.
            evac_cycle = ["v", "a", "v", "v", "a"]

            def supertile(s):
                b0 = s * SUPER
                xt = xt_pool.tile([128, N_CHUNK, SUPER], _BF16, tag="xt")
                nc.sync.dma_start(
                    out=xt[:],
                    in_=xt_d[:, b0 : b0 + SUPER].rearrange(
                        "(c p) b -> p c b", c=N_CHUNK),
                )
                if ablate == "dma_only":
                    st = stage.tile([128, SUPER // 128, N_GROUPS, D], _BF16,
                                    tag="stage")
                    nc.vector.tensor_copy(st[:, 0, 0, 0:4], xt[0:128, 0, 0:4])
                    nc.scalar.dma_start(
                        out=out_d[b0 : b0 + SUPER].rearrange(
                            "(j p) g d -> p j g d", j=SUPER // 128),
                        in_=st[:])
                    return

                # squares (gpsimd) + chunk adds (DVE) -> sqt [128, SUPER]
                sq4 = sq4_pool.tile([128, N_CHUNK, SUPER], _BF16, tag="sq4")
                for c in range(N_CHUNK):
                    nc.gpsimd.tensor_mul(sq4[:, c], xt[:, c], xt[:, c])
                t01 = sqt_pool.tile([128, SUPER], _BF16, tag="sqt")
                nc.gpsimd.tensor_add(t01[:], sq4[:, 0], sq4[:, 1])
                t23 = sqt_pool.tile([128, SUPER], _BF16, tag="sqt")
                nc.gpsimd.tensor_add(t23[:], sq4[:, 2], sq4[:, 3])
                sqt = sqt_pool.tile([128, SUPER], _BF16, tag="sqt")
                nc.gpsimd.tensor_add(sqt[:], t01[:], t23[:])

                ssq = ps_small.tile([1, SUPER], _F32, tag="small")
                nc.tensor.matmul(ssq[:], ones_col, sqt[:], start=True, stop=True)
                # s = exp(-0.5 * ln(tau^2 * ssq)) = 1/(tau*||x||)
                t1 = tiny.tile([1, SUPER], _F32, tag="tiny")
                nc.scalar.activation(t1[:], ssq[:], _LN, scale=float(TAU * TAU))
                s_t = tiny.tile([1, SUPER], cf_dt, tag="tiny")
                nc.scalar.activation(s_t[:], t1[:], _EXP, scale=-0.5)
                # S = broadcast of s to 128 partitions (PE)
                S_ps = ps_small.tile([128, SUPER], _F32, tag="small")
                nc.tensor.matmul(S_ps[:], ones_row, s_t[:], start=True, stop=True)
                S_sb = sp_pool.tile([128, SUPER], _F32, tag="S")
                nc.vector.tensor_copy(S_sb[:], S_ps[:])

                # logits^T for both layouts -> one [128, 2, SUPER] PSUM tile
                raw2 = ps_mid.tile([128, 2, SUPER], _F32, tag="mid")
                for l in range(2):
                    for c in range(N_CHUNK):
                        nc.tensor.matmul(
                            raw2[:, l, :], pnt(c, l), xt[:, c],
                            start=(c == 0), stop=(c == N_CHUNK - 1))
                rawS = rw_pool.tile([128, 2, SUPER], _F32, tag="rawS")
                for l in range(2):
                    nc.vector.tensor_mul(rawS[:, l, :], raw2[:, l, :], S_sb[:])
                et2 = et_pool.tile([128, 2, SUPER], _BF16, tag="et")
                nc.scalar.activation(et2[:], rawS[:], _EXP)

                # segment sums over both layouts -> [5, SUPER]
                sums = ps_small.tile([N_GROUPS, SUPER], _F32, tag="small")
                for l in range(2):
                    nc.tensor.matmul(sums[:], ind_l(l), et2[:, l, :],
                                     start=(l == 0), stop=(l == 1))
                inv_f = tiny.tile([N_GROUPS, SUPER], _F32, tag="tiny")
                nc.vector.reciprocal(inv_f[:], sums[:])
                inv_b = tiny.tile([N_GROUPS, SUPER], _BF16, tag="tiny")
                nc.scalar.copy(inv_b[:], inv_f[:])

                # invB broadcast per layout -> [128, 2, SUPER] PSUM
                invB2 = ps_mid.tile([128, 2, SUPER], _F32, tag="mid")
                for l in range(2):
                    nc.tensor.matmul(invB2[:, l, :], indt_l(l), inv_b[:],
                                     start=True, stop=True)
                qt2 = qt_pool.tile([128, 2, SUPER], _BF16, tag="qt")
                nc.vector.tensor_mul(qt2[:], et2[:], invB2[:])

                # mm2: out[b, g, :] = q_g @ W_g per 128-row tile; small-K
                # matmuls share the PE array via tile_position.
                st = stage.tile([128, SUPER // 128, N_GROUPS, D], _BF16,
                                tag="stage")
                ei = 0
                for j in range(SUPER // 128):
                    jc = slice(j * 128, (j + 1) * 128)
                    for name, (K, l, base) in groups:
                        g = list(GROUP_DIMS).index(name)
                        po = ps_out.tile([128, D], _F32, tag="out")
                        tp = (base, 0) if MM2_TILE_POS else None
                        nc.tensor.matmul(
                            po[:], qt2[base : base + K, l, jc],
                            w_l(l, base, K), start=True, stop=True,
                            tile_position=tp)
                        eng = evac_cycle[ei % len(evac_cycle)]
                        ei += 1
                        dst = st[:, j, g, :]
                        if eng == "v":
                            nc.vector.tensor_copy(dst, po[:])
                        elif eng == "a":
                            nc.scalar.copy(dst, po[:])
                        else:
                            nc.gpsimd.tensor_copy(dst, po[:])
                if ablate != "no_outdma":
                    nc.scalar.dma_start(
                        out=out_d[b0 : b0 + SUPER].rearrange(
                            "(j p) g d -> p j g d", j=SUPER // 128),
                        in_=st[:])

            if loop_k is None:
                for _ in range(repeat):
                    for s in range(N_SUPER):
                        supertile(s)
            else:
                with tc.For_i(0, loop_k, 1):
                    for s in range(N_SUPER):
                        supertile(s)

    return nc


# ------------------------------------------------------------- entry point
def make_core_inputs(inputs):
    """Full inputs dict -> list of 8 per-core input maps."""
    x = np.asarray(inputs["x"], dtype=np.float32)
    assert x.shape == (B, D)
    protos = {k: inputs[f"P_{k}"] for k in GROUP_DIMS}
    cb, cf = build_host_constants(protos)
    maps = []
    for i in range(N_CORES):
        shard = x[i * B_CORE : (i + 1) * B_CORE]
        xt = np.ascontiguousarray(shard.T).astype(_NP_BF16)
        maps.append({"xt": xt, "cb": cb, "cf": cf})
    return maps


_CACHE = {}


def kernel(x, P_gender, P_hair, P_top, P_pants, P_shoes):
    from concourse.bass_utils import run_bass_kernel_spmd

    inputs = dict(x=x, P_gender=P_gender, P_hair=P_hair, P_top=P_top,
                  P_pants=P_pants, P_shoes=P_shoes)
    if "nc" not in _CACHE:
        _CACHE["nc"] = build_program()
    in_maps = make_core_inputs(inputs)
    res = run_bass_kernel_spmd(_CACHE["nc"], in_maps, list(range(N_CORES)))
    return np.concatenate(
        [np.asarray(res.results[i]["out"]).astype(np.float32)
         for i in range(N_CORES)], axis=0)


# revision 11
# speedup vs baseline: 1.0271x; 1.0271x over previous
"""Trainium2 Bass kernel for nn_CerberusSemanticIDBranch (vq_codebook).

Reference semantics (per group g with prototypes P_g [K_g, D]):
    xn = x / (||x|| + 1e-6)
    logits = (xn @ l2(P_g).T) / tau
    q = softmax(logits)
    q_aff = q @ A_g;  q_aff /= (q_aff.sum(-1) + 1e-6)
    out_g = q_aff @ P_g
stacked over 5 groups -> [B, 5, D].

Host folds: A_g row-sums are constant -> out_g = q @ W_g with
W_g = (A_g/(c_g+1e-6)) @ P_g; l2(P) precomputed; 1/(tau*||x||) applied
as a per-batch scale S on the logits.

v2 (PE/DMA rework vs baseline):
  * bf16 streams: xt input, prototype/W constants, et/qt intermediates
    and the output store are bf16 (host upcasts the result to f32).
    Halves HBM traffic and turns the 4-cyc/col f32 matmuls (seg/invb)
    into 1-cyc/col bf16 ones.
  * ssq via a single PE matmul on pre-summed squares (chunk squares on
    gpsimd, chunk adds on DVE): 2048 -> 512 PE cycles per supertile.
  * S broadcast matmul in f32r (1 cyc/col), knob to f32.
  * mm2 uses tile_position so the small-K group matmuls share the PE
    array: A-layout pair at rows (0,64), B-layout triple at (0,32,64).
  * raw/invB PSUM tiles merged per layout-pair ([128,2,512]); PSUM
    pools rebalanced small(1)+mid(2x2)+out(3x1) = 8 banks.
  * input DMA on the SP HWDGE ring, output DMA on the ACT ring; one
    merged output DMA per 512-row supertile.

Data parallel over 8 NeuronCores: core i handles rows [i*4096, (i+1)*4096).
"""

import itertools
import sys

import numpy as np

sys.path.insert(0, "/opt/trn_rl_repo")

import ml_dtypes  # noqa: E402

import concourse.bass as bass  # noqa: E402
import concourse.tile as tile  # noqa: E402
from concourse import mybir  # noqa: E402
from concourse.vector_clock import ScopedClock  # noqa: E402

# ---------------------------------------------------------------- problem
GROUP_DIMS = {
    "gender": [2],
    "hair": [5, 3],
    "top": [8, 5],
    "pants": [8, 5],
    "shoes": [6, 4],
}
TAU = 0.07
B, D = 32768, 512
N_CORES = 8
B_CORE = B // N_CORES          # 4096
SUPER = 512                    # batch rows per supertile
N_SUPER = B_CORE // SUPER      # 8
N_CHUNK = D // 128             # 4
N_GROUPS = 5

# group -> (K, layout, base partition); layout A=0, B=1
GROUP_PLACEMENT = {
    "gender": (2, 1, 0),
    "hair": (15, 1, 32),
    "top": (40, 0, 0),
    "pants": (40, 0, 64),
    "shoes": (24, 1, 64),
}
# mm2 issue order: A-layout pair then B-layout triple (tile_position
# concurrency within each set: disjoint 32-aligned PE array rows).
MM2_ORDER = ["top", "pants", "gender", "hair", "shoes"]

# bf16 const blob column offsets ([128, CB_COLS])
_PNT_OFF = 0                       # 4 chunks x (A 128 | B 128) = 1024
_W_OFF = 1024                      # 2 layouts x 512
_IND_OFF = 2048                    # 2 layouts x 5
_INDT_OFF = 2058                   # 2 layouts x 128 (rows 0:5)
_ONE_OFF = 2314                    # [128, 1] ones
CB_COLS = 2315
CF_COLS = 128                      # f32 ones blob (row 0 = ones_row)

MM2_TILE_POS = True
BCAST_DT = "f32r"                  # "f32r" (1 cyc/col) or "f32" (exact)

_F32 = mybir.dt.float32
_F32R = mybir.dt.float32r
_BF16 = mybir.dt.bfloat16
_EXP = mybir.ActivationFunctionType.Exp
_LN = mybir.ActivationFunctionType.Ln
_NP_BF16 = ml_dtypes.bfloat16


# ------------------------------------------------------------- tile patch
_NOP_ID = [0]


def _spread_all_waits(nc, max_waits=1):
    """This walrus build rejects instructions carrying more than one sync
    wait (setupSyncWait: "Too many sync wait commands").  Rewrite every
    block so extra waits ride on dedicated same-engine NOPs placed just
    before the instruction (engine queues are FIFO, so semantics hold)."""
    for fn in nc.m.functions:
        for blk in fn.blocks:
            insts = list(blk.instructions)
            out = []
            changed = False
            for inst in insts:
                si = inst.sync_info
                waits = list(si.on_wait) if si is not None and si.on_wait else []
                if len(waits) > max_waits:
                    changed = True
                    for w in waits[:-max_waits]:
                        _NOP_ID[0] += 1
                        out.append(
                            mybir.InstNoOp(
                                name=f"waitnop-{_NOP_ID[0]}",
                                engine=inst.engine,
                                bass_nofuse=True,
                                sync_info=mybir.SyncInfo(
                                    on_wait=[w], on_update=[]),
                            ))
                    si.on_wait = waits[-max_waits:]
                out.append(inst)
            if changed:
                blk.instructions = out


def _patched_drain_and_barrier(self, tick_clock, wait_clock):
    probe = self.nc.sync.nop(nofuse=True)
    wait_clock.add_sem_waits(probe.ins, ScopedClock({None: tick_clock.global_clock}))
    drain_inst = self.nc.sync.drain()
    del drain_inst
    self.nc.all_engine_barrier()
    assert self.sems is not None
    popped = self.nc._tile_sem_poison_stack.pop()
    assert popped is self._sem_poison
    self.nc.clear_and_free_semaphores(list(self.sems.allocated().values()))
    self.nc.all_engine_barrier()
    _spread_all_waits(self.nc)


_patched = False


def _install_tile_patch():
    global _patched
    if not _patched:
        tile.TileContext._drain_and_barrier = _patched_drain_and_barrier
        _patched = True


# --------------------------------------------------------- host constants
def _affinity(dims):
    combos = np.array(
        list(itertools.product(*[range(d) for d in dims])), dtype=np.int32
    )
    return (combos[:, None, :] == combos[None, :, :]).mean(-1).astype(np.float64)


def build_host_constants(protos):
    """protos: dict name -> P_g [K_g, D] float32. Returns (cb bf16
    [128, CB_COLS], cf f32 [128, CF_COLS]) shared by all cores."""
    pn_pad = np.zeros((2, 128, D), dtype=np.float32)   # l2-normalized, padded
    w_pad = np.zeros((2, 128, D), dtype=np.float32)    # (A/c) @ P, padded
    ind = np.zeros((2, 128, N_GROUPS), dtype=np.float32)
    indt = np.zeros((2, N_GROUPS, 128), dtype=np.float32)

    for g, name in enumerate(GROUP_DIMS):
        P = np.asarray(protos[name], dtype=np.float32)
        K, layout, base = GROUP_PLACEMENT[name]
        assert P.shape == (K, D)
        norm = np.linalg.norm(P, axis=-1, keepdims=True).astype(np.float32)
        pn = P / (norm + np.float32(1e-6))
        A = _affinity(GROUP_DIMS[name])                 # [K, K] float64
        c = A[0].sum() + 1e-6                            # constant row sum
        W = ((A / c) @ P.astype(np.float64)).astype(np.float32)
        pn_pad[layout, base : base + K] = pn
        w_pad[layout, base : base + K] = W
        ind[layout, base : base + K, g] = 1.0
        indt[layout, g, base : base + K] = 1.0

    cb = np.zeros((128, CB_COLS), dtype=np.float32)
    for c in range(N_CHUNK):
        for l in range(2):
            cb[:, _PNT_OFF + c * 256 + l * 128 : _PNT_OFF + c * 256 + (l + 1) * 128] = (
                pn_pad[l][:, c * 128 : (c + 1) * 128].T
            )
    for l in range(2):
        cb[:, _W_OFF + l * D : _W_OFF + (l + 1) * D] = w_pad[l]
        cb[:, _IND_OFF + l * N_GROUPS : _IND_OFF + (l + 1) * N_GROUPS] = ind[l]
        cb[0:N_GROUPS, _INDT_OFF + l * 128 : _INDT_OFF + (l + 1) * 128] = indt[l]
    cb[:, _ONE_OFF] = 1.0
    cf = np.ones((128, CF_COLS), dtype=np.float32)
    return cb.astype(_NP_BF16), cf


# ------------------------------------------------------------ bass program
def build_program(loop_k=None, ablate=None, repeat=1):
    """Emit the SPMD program. loop_k: if set, wrap the body in a tc.For_i
    repeat for delta-timing. repeat: python-unrolled repeats. ablate:
    None | "dma_only" | "no_outdma" (perf diagnostics; wrong results)."""
    _install_tile_patch()
    nc = bass.Bass("TRN2", target_bir_lowering=False, debug=False,
                   num_devices=N_CORES)
    xt_d = nc.dram_tensor("xt", [D, B_CORE], _BF16, kind="ExternalInput").ap()
    cb_d = nc.dram_tensor("cb", [128, CB_COLS], _BF16,
                          kind="ExternalInput").ap()
    cf_dt = _F32R if BCAST_DT == "f32r" else _F32
    cf_d = nc.dram_tensor("cf", [128, CF_COLS], cf_dt,
                          kind="ExternalInput").ap()
    out_d = nc.dram_tensor("out", [B_CORE, N_GROUPS, D], _BF16,
                           kind="ExternalOutput").ap()

    with tile.TileContext(nc) as tc:
        import contextlib

        with contextlib.ExitStack() as ctx:
            cpool = ctx.enter_context(tc.tile_pool(name="consts", bufs=1))
            xt_pool = ctx.enter_context(tc.tile_pool(name="xt", bufs=4))
            sq4_pool = ctx.enter_context(tc.tile_pool(name="sq4", bufs=2))
            sqt_pool = ctx.enter_context(tc.tile_pool(name="sqt", bufs=4))
            tiny = ctx.enter_context(tc.tile_pool(name="tiny", bufs=6))
            sp_pool = ctx.enter_context(tc.tile_pool(name="sp", bufs=2))
            rw_pool = ctx.enter_context(tc.tile_pool(name="rw", bufs=2))
            et_pool = ctx.enter_context(tc.tile_pool(name="et", bufs=3))
            qt_pool = ctx.enter_context(tc.tile_pool(name="qt", bufs=3))
            stage = ctx.enter_context(tc.tile_pool(name="stage", bufs=3))
            ps_small = ctx.enter_context(
                tc.tile_pool(name="ps_small", bufs=2, space="PSUM"))
            ps_mid = ctx.enter_context(
                tc.tile_pool(name="ps_mid", bufs=2, space="PSUM"))
            ps_out = ctx.enter_context(
                tc.tile_pool(name="ps_out", bufs=2, space="PSUM"))

            cb = cpool.tile([128, CB_COLS], _BF16)
            nc.sync.dma_start(out=cb[:], in_=cb_d[:])
            cf = cpool.tile([128, CF_COLS], cf_dt)
            nc.sync.dma_start(out=cf[:], in_=cf_d[:])

            def pnt(c, l):
                o = _PNT_OFF + c * 256 + l * 128
                return cb[:, o : o + 128]

            def w_l(l, base, K):
                return cb[base : base + K, _W_OFF + l * D : _W_OFF + (l + 1) * D]

            def ind_l(l):
                o = _IND_OFF + l * N_GROUPS
                return cb[:, o : o + N_GROUPS]

            def indt_l(l):
                o = _INDT_OFF + l * 128
                return cb[0:N_GROUPS, o : o + 128]

            ones_col = cb[:, _ONE_OFF : _ONE_OFF + 1]          # [128,1] bf16
            ones_row = cf[0:1, 0:128]                          # [1,128]

            groups = [(name, GROUP_PLACEMENT[name]) for name in MM2_ORDER]
            # evac engine schedule: DVE-heavy (245G vs 153G elem/s);
            # gpsimd cannot read PSUM so it only gets SBUF-side work.
            evac_cycle = ["v", "a", "v", "v", "a"]

            # ---- software-pipelined phases (engine queues are in-order,
            # so emission order decides what an engine can run while a
            # dependent instruction of an older supertile stalls).
            def phA(s):
                """input DMA for supertile s (SP ring)."""
                b0 = s * SUPER
                xt = xt_pool.tile([128, N_CHUNK, SUPER], _BF16, tag="xt")
                nc.sync.dma_start(
                    out=xt[:],
                    in_=xt_d[:, b0 : b0 + SUPER].rearrange(
                        "(c p) b -> p c b", c=N_CHUNK),
                )
                return xt

            def phB(s, xt):
                """norm-scale pipeline: S_sb = 1/(tau*||x||) broadcast."""
                sq4 = sq4_pool.tile([128, N_CHUNK, SUPER], _BF16, tag="sq4")
                for c in range(N_CHUNK):
                    nc.gpsimd.tensor_mul(sq4[:, c], xt[:, c], xt[:, c])
                t01 = sqt_pool.tile([128, SUPER], _BF16, tag="sqt")
                nc.gpsimd.tensor_add(t01[:], sq4[:, 0], sq4[:, 1])
                t23 = sqt_pool.tile([128, SUPER], _BF16, tag="sqt")
                nc.gpsimd.tensor_add(t23[:], sq4[:, 2], sq4[:, 3])
                sqt = sqt_pool.tile([128, SUPER], _BF16, tag="sqt")
                nc.gpsimd.tensor_add(sqt[:], t01[:], t23[:])
                ssq = ps_small.tile([1, SUPER], _F32, tag="small")
                nc.tensor.matmul(ssq[:], ones_col, sqt[:], start=True, stop=True)
                t1 = tiny.tile([1, SUPER], _F32, tag="tiny")
                nc.scalar.activation(t1[:], ssq[:], _LN, scale=float(TAU * TAU))
                s_t = tiny.tile([1, SUPER], cf_dt, tag="tiny")
                nc.scalar.activation(s_t[:], t1[:], _EXP, scale=-0.5)
                S_ps = ps_small.tile([128, SUPER], _F32, tag="small")
                nc.tensor.matmul(S_ps[:], ones_row, s_t[:], start=True, stop=True)
                S_sb = sp_pool.tile([128, SUPER], _F32, tag="S")
                nc.vector.tensor_copy(S_sb[:], S_ps[:])
                return S_sb

            def phC(s, xt):
                """logits^T for both layouts -> [128, 2, SUPER] PSUM."""
                raw2 = ps_mid.tile([128, 2, SUPER], _F32, tag="mid")
                for l in range(2):
                    for c in range(N_CHUNK):
                        nc.tensor.matmul(
                            raw2[:, l, :], pnt(c, l), xt[:, c],
                            start=(c == 0), stop=(c == N_CHUNK - 1))
                return raw2

            def phD(s, raw2, S_sb):
                """softmax + affinity normalization -> qt2 (per layout)."""
                rawS = rw_pool.tile([128, 2, SUPER], _F32, tag="rawS")
                et2 = et_pool.tile([128, 2, SUPER], _BF16, tag="et")
                sums = ps_small.tile([N_GROUPS, SUPER], _F32, tag="small")
                for l in range(2):
                    nc.vector.tensor_mul(rawS[:, l, :], raw2[:, l, :], S_sb[:])
                    nc.scalar.activation(et2[:, l, :], rawS[:, l, :], _EXP)
                    nc.tensor.matmul(sums[:], ind_l(l), et2[:, l, :],
                                     start=(l == 0), stop=(l == 1))
                inv_f = tiny.tile([N_GROUPS, SUPER], _F32, tag="tiny")
                nc.vector.reciprocal(inv_f[:], sums[:])
                inv_b = tiny.tile([N_GROUPS, SUPER], _BF16, tag="tiny")
                nc.scalar.copy(inv_b[:], inv_f[:])
                invB2 = ps_mid.tile([128, 2, SUPER], _F32, tag="mid")
                qt2 = qt_pool.tile([128, 2, SUPER], _BF16, tag="qt")
                for l in range(2):
                    nc.tensor.matmul(invB2[:, l, :], indt_l(l), inv_b[:],
                                     start=True, stop=True)
                    nc.vector.tensor_mul(qt2[:, l, :], et2[:, l, :],
                                         invB2[:, l, :])
                return qt2

            def phE(s, qt2):
                """mm2 (tile_position-concurrent small-K) + evac + out DMA."""
                b0 = s * SUPER
                st = stage.tile([128, SUPER // 128, N_GROUPS, D], _BF16,
                                tag="stage")
                ei = 0
                for j in range(SUPER // 128):
                    jc = slice(j * 128, (j + 1) * 128)
                    for name, (K, l, base) in groups:
                        g = list(GROUP_DIMS).index(name)
                        po = ps_out.tile([128, D], _F32, tag="out")
                        tp = (base, 0) if MM2_TILE_POS else None
                        nc.tensor.matmul(
                            po[:], qt2[base : base + K, l, jc],
                            w_l(l, base, K), start=True, stop=True,
                            tile_position=tp)
                        eng = evac_cycle[ei % len(evac_cycle)]
                        ei += 1
                        dst = st[:, j, g, :]
                        if eng == "v":
                            nc.vector.tensor_copy(dst, po[:])
                        else:
                            nc.scalar.copy(dst, po[:])
                if ablate != "no_outdma":
                    nc.sync.dma_start(
                        out=out_d[b0 : b0 + SUPER].rearrange(
                            "(j p) g d -> p j g d", j=SUPER // 128),
                        in_=st[:])

            def dma_only_tile(s):
                b0 = s * SUPER
                xt = phA(s)
                st = stage.tile([128, SUPER // 128, N_GROUPS, D], _BF16,
                                tag="stage")
                nc.vector.tensor_copy(st[:, 0, 0, 0:4], xt[0:128, 0, 0:4])
                nc.sync.dma_start(
                    out=out_d[b0 : b0 + SUPER].rearrange(
                        "(j p) g d -> p j g d", j=SUPER // 128),
                    in_=st[:])

            def body():
                if ablate == "dma_only":
                    for s in range(N_SUPER):
                        dma_only_tile(s)
                    return
                xts, Ss, raws = {}, {}, {}

                def ensureA(s):
                    if s < N_SUPER and s not in xts:
                        xts[s] = phA(s)

                ensureA(0)
                ensureA(1)
                Ss[0] = phB(0, xts[0])
                raws[0] = phC(0, xts[0])
                for s in range(N_SUPER):
                    ensureA(s + 2)
                    if s + 1 < N_SUPER:
                        Ss[s + 1] = phB(s + 1, xts[s + 1])
                        raws[s + 1] = phC(s + 1, xts[s + 1])
                    qt2 = phD(s, raws.pop(s), Ss.pop(s))
                    phE(s, qt2)

            if loop_k is None:
                for _ in range(repeat):
                    body()
            else:
                with tc.For_i(0, loop_k, 1):
                    for _ in range(repeat):
                        body()

    return nc


# ------------------------------------------------------------- entry point
def make_core_inputs(inputs):
    """Full inputs dict -> list of 8 per-core input maps."""
    x = np.asarray(inputs["x"], dtype=np.float32)
    assert x.shape == (B, D)
    protos = {k: inputs[f"P_{k}"] for k in GROUP_DIMS}
    cb, cf = build_host_constants(protos)
    maps = []
    for i in range(N_CORES):
        shard = x[i * B_CORE : (i + 1) * B_CORE]
        xt = np.ascontiguousarray(shard.T).astype(_NP_BF16)
        maps.append({"xt": xt, "cb": cb, "cf": cf})
    return maps


_CACHE = {}


def kernel(x, P_gender, P_hair, P_top, P_pants, P_shoes):
    from concourse.bass_utils import run_bass_kernel_spmd

    inputs = dict(x=x, P_gender=P_gender, P_hair=P_hair, P_top=P_top,
                  P_pants=P_pants, P_shoes=P_shoes)
    if "nc" not in _CACHE:
        _CACHE["nc"] = build_program()
    in_maps = make_core_inputs(inputs)
    res = run_bass_kernel_spmd(_CACHE["nc"], in_maps, list(range(N_CORES)))
    return np.concatenate(
        [np.asarray(res.results[i]["out"]).astype(np.float32)
         for i in range(N_CORES)], axis=0)
